# revision 25
# baseline (speedup 1.0000x reference)
"""Trainium2 Bass kernel for the three-GEU (text/video/audio) embedding model.

Strategy (8 NeuronCores, one chip):
  - Tensor-parallel column sharding: core c owns output columns [512c, 512(c+1))
    of every linear; it reads only its 1/8 slice of each weight matrix.
  - Weight compression: all weights except Wv ship as int8 (global scale) and
    are cast to fp16 on-device (vector+gpsimd) before the PE consumes them.
    The dequant scale is folded into the host-prescaled activations (GEMM1)
    and into scaled identity matrices used by the h-transposes (GEMM2), so
    the device applies no explicit scales. A rank-1 bias correction
    (calibrated against the batch-mean activations) cancels the coherent
    part of the quantization error.
  - Preprocessing (text max-pool over L, audio ragged masked-mean over T) is
    sharded over the feature dim, computed in transposed layout, and an
    AllGather assembles the full [K, B] activations every core needs.
  - Each GEU: GEMM1 -> PE-transpose h (scaled ident) -> AllGather(hT) ->
    GEMM2 -> sigmoid, y = h * sig(g), partial sum(y^2); AllGather of the
    norm partials (+ local sum) then rsqrt scaling on-device.
  - DMA discipline: all activation DMAs are issued on the two HWDGE rings
    BEFORE any weight chunk, so the pooled activations (and hence the first
    AllGather) are ready ~10us in instead of starving behind weight traffic.
"""

import numpy as np

B = 64
L = 30
D = 4096
DA = 1024
T = 128
NCORES = 8
S = D // NCORES     # 512: per-core output shard of D
SA = DA // NCORES   # 128: per-core shard of Da
KD = D // 128       # 32 k-tiles over D
KA = DA // 128      # 8 k-tiles over Da
CH = 8              # k-tiles per weight DMA chunk (1 MiB fp16 chunks)

INT8_W = ()          # int8+on-device-cast measured slower: casts run at only
                     # ~110 (DVE) / ~37 (GpSimd) G elem/s and starve the
                     # pipeline. Keep all weights fp16.

_STATE: dict = {}


def _build():
    from contextlib import ExitStack

    import concourse.bass as bass
    import concourse.tile as tile
    from concourse import bacc, mybir
    from concourse.bass import ts

    fp16 = mybir.dt.float16
    f32 = mybir.dt.float32
    i8 = mybir.dt.int8
    AX = mybir.AxisListType
    ALU = mybir.AluOpType
    ACTF = mybir.ActivationFunctionType

    nc = bacc.Bacc(
        "TRN2",
        target_bir_lowering=False,
        debug=False,
        enable_asserts=False,
        num_devices=NCORES,
    )
    RG = [list(range(NCORES))]

    # --- kernel I/O (per-core shards, staged by the host wrapper) ---
    w_in = {}
    for name, kk in [("wt", D), ("wgt", D), ("wv", D), ("wgv", D),
                     ("wga", D), ("wa", DA)]:
        nkt = kk // 128
        nch = max(1, nkt // CH)
        dt = i8 if name in INT8_W else fp16
        w_in[name] = nc.dram_tensor(
            name, [nch, 128, (nkt // nch) * S], dt, kind="ExternalInput")
    textT = nc.dram_tensor("textT", [S, B, L], fp16, kind="ExternalInput")
    audioT = nc.dram_tensor("audioT", [T, B, SA], fp16, kind="ExternalInput")
    vT_d = nc.dram_tensor("vT", [128, KD, B], fp16, kind="ExternalInput")
    maskT_d = nc.dram_tensor("maskT", [T, B], fp16, kind="ExternalInput")
    biases_d = nc.dram_tensor("biases", [1, 6 * S], fp16, kind="ExternalInput")
    idents_d = nc.dram_tensor("idents", [B, 3 * B], fp16, kind="ExternalInput")
    EMBEDS = ("text", "video", "audio")
    out_d = {
        e: nc.dram_tensor(f"out_{e}", [B, S], f32, kind="ExternalOutput")
        for e in EMBEDS
    }

    BIAS_IDX = {("text", 1): 0, ("text", 2): 1, ("video", 1): 2,
                ("video", 2): 3, ("audio", 1): 4, ("audio", 2): 5}

    with ExitStack() as ctx:
        tc = ctx.enter_context(tile.TileContext(nc))

        persist = ctx.enter_context(tc.tile_pool(name="persist", bufs=1))
        wpool = ctx.enter_context(tc.tile_pool(name="wstream", bufs=16))
        w8pool = ctx.enter_context(tc.tile_pool(name="w8stream", bufs=2))
        work = ctx.enter_context(tc.tile_pool(name="work", bufs=2))
        psum = ctx.enter_context(tc.tile_pool(name="psum", bufs=2, space="PSUM"))
        dram = ctx.enter_context(tc.tile_pool(name="dram", bufs=1, space="DRAM"))

        # ---- persistent SBUF tiles ----
        # Five pipelined AllGathers (ncfw runs them FIFO; each one's wire
        # time hides under PE work): AG1a pooled acts -> AG1b video hT ->
        # AG2a audio hT -> AG2t text hT -> AGn norm partials.
        acts_all = persist.tile([128, NCORES, 5, B], fp16)
        au_sb = persist.tile([T, B, SA], fp16)
        vt_sb = persist.tile([128, KD, B], fp16)
        msk_sb = persist.tile([T, B], fp16)
        bias_sb = persist.tile([1, 6, S], fp16)
        ones_sb = persist.tile([1, B], fp16)
        idents_sb = persist.tile([B, 3, B], fp16)
        stg = persist.tile([128, 5, B], fp16)
        nsq = persist.tile([B, 4], f32)
        nsqg = persist.tile([B, NCORES, 3], f32)
        nsum = persist.tile([B, 3], f32)
        nrm = persist.tile([B, 3], f32)
        rcp = persist.tile([B, 3], f32)
        hstg = {e: persist.tile([128, 4, B], fp16, name=f"hstg_{e}")
                for e in EMBEDS}
        hTg = {e: persist.tile([128, NCORES, 4, B], fp16, name=f"hTg_{e}")
               for e in EMBEDS}
        h16 = {e: persist.tile([B, S], fp16, name=f"h16_{e}") for e in EMBEDS}
        y_sb = {e: persist.tile([B, S], f32, name=f"y_{e}") for e in EMBEDS}

        nc.gpsimd.memset(ones_sb[:], 1.0)
        nc.vector.memset(nsq[:], 0.0)

        # ---- activation DMAs FIRST, split across all three DMA rings.
        # The first collective starts only when the LAST-launched rank
        # (ranks start ~5us apart) has finished its pre-AG1 phase, so the
        # stg-path inputs (audio, text, mask) get absolute priority.
        au_view = audioT.ap().rearrange("(h p) b c -> h p b c", h=2)
        nc.sync.dma_start(au_sb[0:64, :, :], au_view[0])
        nc.scalar.dma_start(au_sb[64:128, :, :], au_view[1])
        nc.gpsimd.dma_start(msk_sb[:], maskT_d.ap())
        t_view = textT.ap().rearrange("(n p) b l -> n p b l", p=128)
        # ---- text max-pool over L (sharded over d) -> stg[:, 0:4, :] ----
        for i in range(4):
            txt = work.tile([128, B, L], fp16, name="tx", tag="tx")
            eng = nc.sync if i % 2 == 0 else nc.scalar
            eng.dma_start(txt[:], t_view[i])
            nc.vector.reduce_max(stg[:, i, :], txt[:], AX.X)
        nc.sync.dma_start(bias_sb[0:1, :, :], biases_d.ap())
        nc.scalar.dma_start(
            idents_sb.rearrange("p e b -> p (e b)"), idents_d.ap())
        nc.scalar.dma_start(vt_sb[:], vT_d.ap())

        # ---- audio ragged masked-mean (sharded over Da): 64 PE matvecs ----
        aT_ps = psum.tile([SA, B], f32, bufs=1)
        for b in range(B):
            nc.tensor.matmul(
                aT_ps[:, b:b + 1], au_sb[:, b, :], msk_sb[:, b:b + 1],
                start=True, stop=True)
        nc.vector.tensor_copy(stg[:, 4, :], aT_ps[:])

        hwdge = [nc.sync, nc.scalar]
        chunk_no = [0]

        def gemm(out_ps, w_dram, n_kt, lhs_fn, bias_idx):
            # bias as a K=1 matmul row; also opens the accumulation group
            nc.tensor.matmul(out_ps[:], ones_sb[:], bias_sb[:, bias_idx, :],
                             start=True, stop=False)
            nch = w_dram.shape[0]
            cnt = n_kt // nch
            for ch in range(nch):
                eng = hwdge[chunk_no[0] % 2]
                chunk_no[0] += 1
                w = wpool.tile([128, cnt, S], fp16, name="wchunk",
                               tag="wchunk")
                eng.dma_start(
                    w[:],
                    w_dram.ap()[ch].rearrange("p (a n) -> p a n", n=S))
                for a in range(cnt):
                    k = ch * cnt + a
                    nc.tensor.matmul(out_ps[:], lhs_fn(k), w[:, a, :],
                                     start=False, stop=(k == n_kt - 1))

        EIDX = {e: i for i, e in enumerate(EMBEDS)}

        def transposes(e, dst):
            # transpose h shard via plain matmul (NOT is_transpose: that
            # path ignores the identity's values); the identity's diagonal
            # carries the GEMM2 dequant scale when int8 weights are in play
            ei = EIDX[e]
            hT_ps = psum.tile([128, 4, B], f32, name="hT_ps", tag="hT_ps",
                              bufs=1)
            for j in range(4):
                nc.tensor.matmul(hT_ps[:, j, :], h16[e][:, ts(j, 128)],
                                 idents_sb[:, ei, :], start=True, stop=True)
            nc.vector.tensor_copy(dst, hT_ps[:])

        def glu_tail(e, g_ps):
            ei = EIDX[e]
            sg16 = work.tile([B, S], fp16, name="sg16", tag="sg16")
            nc.scalar.activation(sg16[:], g_ps[:], ACTF.Sigmoid)
            nc.vector.tensor_mul(y_sb[e][:], h16[e][:], sg16[:])
            ysq = work.tile([B, S], f32, name="ysq", tag="ysq")
            nc.vector.tensor_mul(ysq[:], y_sb[e][:], y_sb[e][:])
            nc.vector.reduce_sum(nsq[:, ei:ei + 1], ysq[:], AX.X)

        def allgather(name, src_sb, dst_sb, n_free):
            # stage SBUF -> DRAM, collective, reload rank-major into SBUF
            cin = dram.tile([128, n_free], fp16, name=f"{name}_in")
            cout = dram.tile([128 * NCORES, n_free], fp16,
                             addr_space="Shared", name=f"{name}_out")
            nc.gpsimd.dma_start(cin[:], src_sb)
            nc.gpsimd.collective_compute(
                "AllGather", ALU.bypass, replica_groups=RG,
                ins=[cin.opt()], outs=[cout.opt()])
            nc.gpsimd.dma_start(
                dst_sb, cout.rearrange("(r p) x -> p r x", p=128))

        # ---- video GEMM1 pre-AG1 (needs only local acts) ----
        h_ps_v = psum.tile([B, S], f32, name="h_ps", tag="h_ps")
        gemm(h_ps_v, w_in["wv"], KD, lambda k: vt_sb[:, k, :],
             BIAS_IDX[("video", 1)])
        nc.vector.tensor_copy(h16["video"][:], h_ps_v[:])
        transposes("video", hstg["video"][:])

        # ---- AG1a: pooled text+audio acts (first sync point; starts as
        # soon as the last-launched rank finishes its act DMAs+pooling) ----
        allgather("ag1a", stg[:], acts_all.rearrange("p r s b -> p r (s b)"),
                  5 * B)
        # ---- AG1b: video hT (ready pre-AG1a; wire hides under GEMM1s) ----
        allgather("ag1b", hstg["video"][:],
                  hTg["video"].rearrange("p r j b -> p r (j b)"), 4 * B)

        def lhs_text(k):
            return acts_all[:, k // 4, k % 4, :]

        def lhs_audio(k):
            return acts_all[:, k, 4, :]

        # ---- audio then text GEMM1 (audio is short: its hT gather can
        # launch early and hide under text GEMM1 + GEMM2-video) ----
        for e, wname, nkt, lf in (("audio", "wa", KA, lhs_audio),
                                  ("text", "wt", KD, lhs_text)):
            h_ps = psum.tile([B, S], f32, name="h_ps", tag="h_ps")
            gemm(h_ps, w_in[wname], nkt, lf, BIAS_IDX[(e, 1)])
            nc.vector.tensor_copy(h16[e][:], h_ps[:])
            transposes(e, hstg[e][:])
            allgather(f"ag2{e[0]}", hstg[e][:],
                      hTg[e].rearrange("p r j b -> p r (j b)"), 4 * B)

        # ---- gating GEMMs; video first (its hT arrived with AG1b) ----
        for e, wname in (("video", "wgv"), ("audio", "wga"),
                         ("text", "wgt")):
            g_ps = psum.tile([B, S], f32, name="g_ps", tag="g_ps")
            gemm(g_ps, w_in[wname], KD,
                 lambda k, e=e: hTg[e][:, k // 4, k % 4, :],
                 BIAS_IDX[(e, 2)])
            glu_tail(e, g_ps)

        # ---- AllGather norm partials; sum locally; normalize; write out ----
        ar_in = dram.tile([B, 3], f32)
        ar_out = dram.tile([B * NCORES, 3], f32, addr_space="Shared")
        nc.gpsimd.dma_start(ar_in[:], nsq[:, 0:3])
        nc.gpsimd.collective_compute(
            "AllGather", ALU.bypass, replica_groups=RG,
            ins=[ar_in.opt()], outs=[ar_out.opt()])
        nc.gpsimd.dma_start(
            nsqg[:], ar_out.rearrange("(r p) x -> p r x", p=B))
        nc.vector.tensor_add(nsum[:], nsqg[:, 0, :], nsqg[:, 1, :])
        for r in range(2, NCORES):
            nc.vector.tensor_add(nsum[:], nsum[:], nsqg[:, r, :])
        nc.scalar.sqrt(nrm[:], nsum[:])
        nc.vector.tensor_scalar_max(nrm[:], nrm[:], 1e-12)
        nc.vector.reciprocal(rcp[:], nrm[:])
        for e in EMBEDS:
            ei = EIDX[e]
            yo = work.tile([B, S], f32, name="yo", tag="yo")
            nc.vector.tensor_scalar_mul(yo[:], y_sb[e][:],
                                        rcp[:, ei:ei + 1])
            nc.sync.dma_start(out_d[e].ap(), yo[:])

    nc.compile()
    return nc


def _get_nc():
    if "nc" not in _STATE:
        _STATE["nc"] = _build()
    return _STATE["nc"]


def _quant_i8(W):
    """Symmetric int8 with a single global scale."""
    Wf = np.asarray(W, np.float32)
    s = float(np.max(np.abs(Wf))) / 127.0
    if s == 0.0:
        s = 1.0
    w8 = np.round(Wf / s).clip(-127, 127).astype(np.int8)
    return w8, s


def _prep_inputs(text, video, audio_feats, Wt, bt, Wgt, bgt, Wv, bv, Wgv, bgv,
                 Wa, ba, Wga, bga, nframes, raw_audio_len):
    """Quantize weights, calibrate biases, shard + transpose into in_maps."""
    f16 = np.float16
    text = np.asarray(text, dtype=np.float32)
    video = np.asarray(video, dtype=np.float32)
    audio = np.asarray(audio_feats, dtype=np.float32)
    Wt = np.asarray(Wt, np.float32)
    Wgt = np.asarray(Wgt, np.float32)
    Wv = np.asarray(Wv, np.float32)
    Wgv = np.asarray(Wgv, np.float32)
    Wa = np.asarray(Wa, np.float32)
    Wga = np.asarray(Wga, np.float32)
    bt = np.asarray(bt, np.float32)
    bgt = np.asarray(bgt, np.float32)
    bv = np.asarray(bv, np.float32)
    bgv = np.asarray(bgv, np.float32)
    ba = np.asarray(ba, np.float32)
    bga = np.asarray(bga, np.float32)

    ratio = int(round(float(np.asarray(raw_audio_len)) / T))
    nf = np.maximum(
        1, (np.asarray(nframes).astype(np.float32) / ratio).astype(np.int32))
    mask = (np.arange(T)[None, :] < nf[:, None]).astype(np.float32)
    mask = mask / nf[:, None].astype(np.float32)          # [B, T] mask/nf
    maskT = np.ascontiguousarray(mask.T).astype(f16)      # [T, B]

    # -- weight quantization (int8 only for names in INT8_W) --
    def quant(Wm, name):
        if name in INT8_W:
            return _quant_i8(Wm)
        return Wm, 1.0

    wt8, s_t = quant(Wt, "wt")
    wa8, s_a = quant(Wa, "wa")
    wv8, s_v = quant(Wv, "wv")
    wgt8, s_gt = quant(Wgt, "wgt")
    wgv8, s_gv = quant(Wgv, "wgv")
    wga8, s_ga = quant(Wga, "wga")
    # GEMM2 scales ride on the transpose identities as fp16 — use the
    # rounded values for the bias calibration below
    s_gt_e = float(np.float16(s_gt))
    s_gv_e = float(np.float16(s_gv))
    s_ga_e = float(np.float16(s_ga))

    # -- calibration: cancel the coherent (batch-mean) quantization error --
    if INT8_W:
        pooled_text = np.max(text, axis=1)                    # [B, D]
        pooled_audio = np.einsum('bct,bt->bc', audio, mask)   # [B, Da]
        xbar_t = pooled_text.mean(0)
        xbar_v = video.mean(0)
        xbar_a = pooled_audio.mean(0)

        def comp(b, wq, s, W, xb):
            if s == 1.0:
                return b
            return b - (wq.astype(np.float32) * s - W) @ xb

        b_eff_t = comp(bt, wt8, s_t, Wt, xbar_t)
        b_eff_v = comp(bv, wv8, s_v, Wv, xbar_v)
        b_eff_a = comp(ba, wa8, s_a, Wa, xbar_a)
        hbar_t = xbar_t @ Wt.T + bt
        hbar_v = xbar_v @ Wv.T + bv
        hbar_a = xbar_a @ Wa.T + ba
        bg_eff_t = comp(bgt, wgt8, s_gt_e, Wgt, hbar_t)
        bg_eff_v = comp(bgv, wgv8, s_gv_e, Wgv, hbar_v)
        bg_eff_a = comp(bga, wga8, s_ga_e, Wga, hbar_a)
    else:
        b_eff_t, b_eff_v, b_eff_a = bt, bv, ba
        bg_eff_t, bg_eff_v, bg_eff_a = bgt, bgv, bga

    # -- activations: GEMM1 dequant scales fold into the acts themselves --
    textT_f = text.transpose(2, 0, 1)
    if s_t != 1.0:
        textT_f = textT_f * s_t
    vT = np.ascontiguousarray(
        video.T.reshape(KD, 128, B).transpose(1, 0, 2)).astype(f16)

    idents = np.zeros((B, 3, B), np.float32)
    idents[:, 0, :] = np.eye(B) * s_gt_e
    idents[:, 1, :] = np.eye(B) * s_gv_e
    idents[:, 2, :] = np.eye(B) * s_ga_e
    idents = idents.reshape(B, 3 * B).astype(f16)

    def wtile(Wq, sl, name):
        wtr = Wq[sl, :].T
        kk = wtr.shape[0]
        nkt = kk // 128
        nch = max(1, nkt // CH)
        cnt = nkt // nch
        dtype = np.int8 if name in INT8_W else f16
        return np.ascontiguousarray(
            wtr.reshape(nch, cnt, 128, S).transpose(0, 2, 1, 3)
            .reshape(nch, 128, cnt * S)).astype(dtype)

    in_maps = []
    for c in range(NCORES):
        sl = slice(c * S, (c + 1) * S)
        sla = slice(c * SA, (c + 1) * SA)
        au_sl = audio[:, sla, :]
        if s_a != 1.0:
            au_sl = au_sl * s_a
        m = {
            "wt": wtile(wt8, sl, "wt"),
            "wgt": wtile(wgt8, sl, "wgt"),
            "wv": wtile(wv8, sl, "wv"),
            "wgv": wtile(wgv8, sl, "wgv"),
            "wga": wtile(wga8, sl, "wga"),
            "wa": wtile(wa8, sl, "wa"),
            "textT": np.ascontiguousarray(textT_f[sl]).astype(f16),
            "audioT": np.ascontiguousarray(
                au_sl.transpose(2, 0, 1)).astype(f16),
            "vT": vT,
            "maskT": maskT,
            "idents": idents,
            "biases": np.stack([
                b_eff_t[sl], bg_eff_t[sl], b_eff_v[sl], bg_eff_v[sl],
                b_eff_a[sl], bg_eff_a[sl],
            ]).reshape(1, -1).astype(f16),
        }
        in_maps.append(m)
    return in_maps


def kernel(text, video, audio_feats, Wt, bt, Wgt, bgt, Wv, bv, Wgv, bgv,
           Wa, ba, Wga, bga, nframes, raw_audio_len):
    from concourse.bass_utils import run_bass_kernel_spmd

    nc = _get_nc()
    in_maps = _prep_inputs(text, video, audio_feats, Wt, bt, Wgt, bgt,
                           Wv, bv, Wgv, bgv, Wa, ba, Wga, bga,
                           nframes, raw_audio_len)
    res = run_bass_kernel_spmd(nc, in_maps, list(range(NCORES)))
    _STATE["last_results"] = res
    outs = []
    for e in ("text", "video", "audio"):
        outs.append(np.concatenate(
            [res.results[c][f"out_{e}"] for c in range(NCORES)], axis=1))
    return tuple(outs)


# revision 35
# speedup vs baseline: 1.1208x; 1.1208x over previous
"""Trainium2 Bass kernel for the three-GEU (text/video/audio) embedding model.

Strategy (8 NeuronCores, one chip):
  - Tensor-parallel column sharding: core c owns output columns [512c, 512(c+1))
    of every linear; it reads only its 1/8 slice of each weight matrix
    (21 MiB fp16 per core — the per-core HBM roofline).
  - Preprocessing (text max-pool over L, audio ragged masked-mean over T) is
    sharded over the feature dim, computed in transposed layout.
  - Five pipelined AllGathers, ordered so each one's wire time hides under
    PE work: AG1a (pooled text+audio acts) -> AG1b (video hT, computed
    pre-AG1a from local acts) -> AG2a (audio hT) -> AG2t (text hT) ->
    AGn (norm partials, summed locally — cheaper than an AllReduce).
    The first collective starts only when the LAST-launched rank (ranks
    start ~5us apart) finishes its act DMAs, so the stg-path inputs get
    absolute ring priority and all weight traffic queues behind them.
  - Each GEU: GEMM1 -> h-transpose via plain matmul against an identity
    (is_transpose ignores the identity's values; the identity's diagonal
    can carry a dequant scale) -> AllGather(hT) -> GEMM2 -> sigmoid,
    y = h * sig(g), partial sum(y^2), gather+sum, rsqrt scaling on-device.
  - INT8_W can flip weights to int8 wire format with host-side bias
    calibration; measured slower on this part (DVE/GpSimd casts run at
    ~110/~37 G elem/s and starve the pipeline), so it ships all-fp16.
"""

import numpy as np

B = 64
L = 30
D = 4096
DA = 1024
T = 128
NCORES = 8
S = D // NCORES     # 512: per-core output shard of D
SA = DA // NCORES   # 128: per-core shard of Da
KD = D // 128       # 32 k-tiles over D
KA = DA // 128      # 8 k-tiles over Da
CH = 8              # k-tiles per weight DMA chunk (1 MiB fp16 chunks)

INT8_W = ()          # int8+on-device-cast measured slower: casts run at only
                     # ~110 (DVE) / ~37 (GpSimd) G elem/s and starve the
                     # pipeline. Keep all weights fp16.

_STATE: dict = {}


def _build():
    from contextlib import ExitStack

    import concourse.bass as bass
    import concourse.tile as tile
    from concourse import bacc, mybir
    from concourse.bass import ts

    fp16 = mybir.dt.float16
    f32 = mybir.dt.float32
    i8 = mybir.dt.int8
    AX = mybir.AxisListType
    ALU = mybir.AluOpType
    ACTF = mybir.ActivationFunctionType

    nc = bacc.Bacc(
        "TRN2",
        target_bir_lowering=False,
        debug=False,
        enable_asserts=False,
        num_devices=NCORES,
    )
    RG = [list(range(NCORES))]

    # --- kernel I/O (per-core shards, staged by the host wrapper) ---
    w_in = {}
    for name, kk in [("wt", D), ("wgt", D), ("wv", D), ("wgv", D),
                     ("wga", D), ("wa", DA)]:
        nkt = kk // 128
        nch = max(1, nkt // CH)
        dt = i8 if name in INT8_W else fp16
        w_in[name] = nc.dram_tensor(
            name, [nch, 128, (nkt // nch) * S], dt, kind="ExternalInput")
    textT = nc.dram_tensor("textT", [S, B, L], fp16, kind="ExternalInput")
    audioT = nc.dram_tensor("audioT", [T, B, SA], fp16, kind="ExternalInput")
    vT_d = nc.dram_tensor("vT", [128, KD, B], fp16, kind="ExternalInput")
    maskT_d = nc.dram_tensor("maskT", [T, B], fp16, kind="ExternalInput")
    biases_d = nc.dram_tensor("biases", [1, 6 * S], fp16, kind="ExternalInput")
    idents_d = nc.dram_tensor("idents", [B, 3 * B], fp16, kind="ExternalInput")
    EMBEDS = ("text", "video", "audio")
    out_d = {
        e: nc.dram_tensor(f"out_{e}", [B, S], f32, kind="ExternalOutput")
        for e in EMBEDS
    }

    BIAS_IDX = {("text", 1): 0, ("text", 2): 1, ("video", 1): 2,
                ("video", 2): 3, ("audio", 1): 4, ("audio", 2): 5}

    with ExitStack() as ctx:
        tc = ctx.enter_context(tile.TileContext(nc))

        persist = ctx.enter_context(tc.tile_pool(name="persist", bufs=1))
        wpool = ctx.enter_context(tc.tile_pool(name="wstream", bufs=11))
        w8pool = ctx.enter_context(tc.tile_pool(name="w8stream", bufs=2))
        work = ctx.enter_context(tc.tile_pool(name="work", bufs=2))
        psum = ctx.enter_context(tc.tile_pool(name="psum", bufs=2, space="PSUM"))
        dram = ctx.enter_context(tc.tile_pool(name="dram", bufs=1, space="DRAM"))

        # ---- persistent SBUF tiles ----
        # Five pipelined AllGathers (ncfw runs them FIFO; each one's wire
        # time hides under PE work): AG1a pooled acts -> AG1b video hT ->
        # AG2a audio hT -> AG2t text hT -> AGn norm partials.
        acts_all = persist.tile([128, NCORES, 5, B], fp16)
        au_sb = persist.tile([T, B, SA], fp16)
        vt_sb = persist.tile([128, KD, B], fp16)
        msk_sb = persist.tile([T, B], fp16)
        bias_sb = persist.tile([1, 6, S], fp16)
        ones_sb = persist.tile([1, B], fp16)
        idents_sb = persist.tile([B, 3, B], fp16)
        stg = persist.tile([128, 5, B], fp16)
        nsq = persist.tile([B, 4], f32)
        nsqg = persist.tile([B, NCORES, 3], f32)
        nsum = persist.tile([B, 3], f32)
        nrm = persist.tile([B, 3], f32)
        rcp = persist.tile([B, 3], f32)
        hstg = {e: persist.tile([128, 4, B], fp16, name=f"hstg_{e}")
                for e in EMBEDS}
        hTg = {e: persist.tile([128, NCORES, 4, B], fp16, name=f"hTg_{e}")
               for e in EMBEDS}
        h16 = {e: persist.tile([B, S], fp16, name=f"h16_{e}") for e in EMBEDS}
        y_sb = {e: persist.tile([B, S], f32, name=f"y_{e}") for e in EMBEDS}

        nc.gpsimd.memset(ones_sb[:], 1.0)
        nc.vector.memset(nsq[:], 0.0)

        # ---- activation DMAs FIRST, split across all three DMA rings.
        # The first collective starts only when the LAST-launched rank
        # (ranks start ~5us apart) has finished its pre-AG1 phase, so the
        # stg-path inputs (audio, text, mask) get absolute priority.
        au_view = audioT.ap().rearrange("(h p) b c -> h p b c", h=2)
        nc.sync.dma_start(au_sb[0:64, :, :], au_view[0])
        nc.scalar.dma_start(au_sb[64:128, :, :], au_view[1])
        nc.gpsimd.dma_start(msk_sb[:], maskT_d.ap())
        t_view = textT.ap().rearrange("(n p) b l -> n p b l", p=128)
        # ---- text max-pool over L (sharded over d) -> stg[:, 0:4, :] ----
        for i in range(4):
            txt = work.tile([128, B, L], fp16, name="tx", tag="tx")
            eng = nc.sync if i % 2 == 0 else nc.scalar
            eng.dma_start(txt[:], t_view[i])
            nc.vector.reduce_max(stg[:, i, :], txt[:], AX.X)
        nc.sync.dma_start(bias_sb[0:1, :, :], biases_d.ap())
        nc.scalar.dma_start(
            idents_sb.rearrange("p e b -> p (e b)"), idents_d.ap())
        nc.scalar.dma_start(vt_sb[:], vT_d.ap())

        # ---- audio ragged masked-mean (sharded over Da): 64 PE matvecs ----
        aT_ps = psum.tile([SA, B], f32, bufs=1)
        for b in range(B):
            nc.tensor.matmul(
                aT_ps[:, b:b + 1], au_sb[:, b, :], msk_sb[:, b:b + 1],
                start=True, stop=True)
        nc.vector.tensor_copy(stg[:, 4, :], aT_ps[:])

        hwdge = [nc.sync, nc.scalar]
        chunk_no = [0]

        def fetch_w(w_dram, n_kt, tag="wchunk"):
            # issue the chunk DMAs now (ring-FIFO position = emission
            # order). wt/wa use dedicated tags: they stay resident until
            # the post-AG1 GEMM1s, and recycling their buffers would stall
            # late gating chunks (and the stage DMAs queued behind them).
            nch = w_dram.shape[0]
            cnt = n_kt // nch
            tiles = []
            nbuf = {"wt": 4, "wa": 1}.get(tag)
            for ch in range(nch):
                eng = hwdge[chunk_no[0] % 2]
                chunk_no[0] += 1
                if nbuf is None:
                    w = wpool.tile([128, cnt, S], fp16, name=tag, tag=tag)
                else:
                    w = wpool.tile([128, cnt, S], fp16, name=tag, tag=tag,
                                   bufs=nbuf)
                eng.dma_start(
                    w[:],
                    w_dram.ap()[ch].rearrange("p (a n) -> p a n", n=S))
                tiles.append(w)
            return tiles, cnt

        def gemm_mms(out_ps, tiles, cnt, n_kt, lhs_fn, bias_idx):
            # bias as a K=1 matmul row; also opens the accumulation group
            nc.tensor.matmul(out_ps[:], ones_sb[:], bias_sb[:, bias_idx, :],
                             start=True, stop=False)
            for ch, w in enumerate(tiles):
                for a in range(cnt):
                    k = ch * cnt + a
                    nc.tensor.matmul(out_ps[:], lhs_fn(k), w[:, a, :],
                                     start=False, stop=(k == n_kt - 1))

        def gemm(out_ps, w_dram, n_kt, lhs_fn, bias_idx, tag="wchunk"):
            tiles, cnt = fetch_w(w_dram, n_kt, tag)
            gemm_mms(out_ps, tiles, cnt, n_kt, lhs_fn, bias_idx)

        EIDX = {e: i for i, e in enumerate(EMBEDS)}

        def transposes(e, dst):
            # transpose h shard via plain matmul (NOT is_transpose: that
            # path ignores the identity's values); the identity's diagonal
            # carries the GEMM2 dequant scale when int8 weights are in play
            ei = EIDX[e]
            hT_ps = psum.tile([128, 4, B], f32, name="hT_ps", tag="hT_ps",
                              bufs=1)
            for j in range(4):
                nc.tensor.matmul(hT_ps[:, j, :], h16[e][:, ts(j, 128)],
                                 idents_sb[:, ei, :], start=True, stop=True)
            nc.vector.tensor_copy(dst, hT_ps[:])

        def glu_tail(e, g_ps):
            ei = EIDX[e]
            sg16 = work.tile([B, S], fp16, name="sg16", tag="sg16")
            nc.scalar.activation(sg16[:], g_ps[:], ACTF.Sigmoid)
            nc.vector.tensor_mul(y_sb[e][:], h16[e][:], sg16[:])
            ysq = work.tile([B, S], f32, name="ysq", tag="ysq")
            nc.vector.tensor_mul(ysq[:], y_sb[e][:], y_sb[e][:])
            nc.vector.reduce_sum(nsq[:, ei:ei + 1], ysq[:], AX.X)

        def allgather(name, src_sb, dst_sb, n_free):
            # stage SBUF -> DRAM, collective, reload rank-major into SBUF
            cin = dram.tile([128, n_free], fp16, name=f"{name}_in")
            cout = dram.tile([128 * NCORES, n_free], fp16,
                             addr_space="Shared", name=f"{name}_out")
            nc.gpsimd.dma_start(cin[:], src_sb)
            nc.gpsimd.collective_compute(
                "AllGather", ALU.bypass, replica_groups=RG,
                ins=[cin.opt()], outs=[cout.opt()])
            nc.gpsimd.dma_start(
                dst_sb, cout.rearrange("(r p) x -> p r x", p=128))

        # ---- video GEMM1 pre-AG1 (needs only local acts) ----
        h_ps_v = psum.tile([B, S], f32, name="h_ps", tag="h_ps")
        gemm(h_ps_v, w_in["wv"], KD, lambda k: vt_sb[:, k, :],
             BIAS_IDX[("video", 1)])
        nc.vector.tensor_copy(h16["video"][:], h_ps_v[:])
        transposes("video", hstg["video"][:])

        # ---- AG1a: pooled text+audio acts (first sync point; starts as
        # soon as the last-launched rank finishes its act DMAs+pooling) ----
        allgather("ag1a", stg[:], acts_all.rearrange("p r s b -> p r (s b)"),
                  5 * B)
        # ---- AG1b: video hT (ready pre-AG1a; wire hides under GEMM1s) ----
        allgather("ag1b", hstg["video"][:],
                  hTg["video"].rearrange("p r j b -> p r (j b)"), 4 * B)

        def lhs_text(k):
            return acts_all[:, k // 4, k % 4, :]

        def lhs_audio(k):
            return acts_all[:, k, 4, :]

        # ---- audio then text GEMM1 (audio is short: its hT gather can
        # launch early and hide under text GEMM1 + GEMM2-video) ----
        for e, wname, nkt, lf in (("audio", "wa", KA, lhs_audio),
                                  ("text", "wt", KD, lhs_text)):
            h_ps = psum.tile([B, S], f32, name="h_ps", tag="h_ps")
            gemm(h_ps, w_in[wname], nkt, lf, BIAS_IDX[(e, 1)], tag=wname)
            nc.vector.tensor_copy(h16[e][:], h_ps[:])
            transposes(e, hstg[e][:])

        # ---- prefetch ALL gating-GEMM weight chunks now, so the stage
        # DMAs below sit behind them in the HWDGE ring FIFOs ----
        g2w = {}
        for e, wname in (("video", "wgv"), ("audio", "wga"),
                         ("text", "wgt")):
            g2w[e] = fetch_w(w_in[wname], KD)

        # ---- late collectives: stage/reload on the (now idle) HWDGE
        # rings — the gpsimd SWDGE queue added ~6-10us of latency per
        # collective handoff. Triggers stay on gpsimd in FIFO order. ----
        ag2 = {}
        for e in ("audio", "text"):
            cin = dram.tile([128, 4 * B], fp16, name=f"ag2{e[0]}_in")
            cout = dram.tile([128 * NCORES, 4 * B], fp16,
                             addr_space="Shared", name=f"ag2{e[0]}_out")
            ag2[e] = (cin, cout)
        ar_in = dram.tile([B, 3], f32)
        ar_out = dram.tile([B * NCORES, 3], f32, addr_space="Shared")

        nc.sync.dma_start(ag2["audio"][0][:], hstg["audio"][:])
        nc.scalar.dma_start(ag2["text"][0][:], hstg["text"][:])
        for e in ("audio", "text"):
            cin, cout = ag2[e]
            nc.gpsimd.collective_compute(
                "AllGather", ALU.bypass, replica_groups=RG,
                ins=[cin.opt()], outs=[cout.opt()])
            eng = nc.scalar if e == "audio" else nc.sync
            eng.dma_start(
                hTg[e].rearrange("p r j b -> p r (j b)"),
                cout.rearrange("(r p) x -> p r x", p=128))

        # ---- gating GEMMs; video first (its hT arrived with AG1b) ----
        for e in ("video", "audio", "text"):
            tiles, cnt = g2w[e]
            g_ps = psum.tile([B, S], f32, name="g_ps", tag="g_ps")
            gemm_mms(g_ps, tiles, cnt, KD,
                     lambda k, e=e: hTg[e][:, k // 4, k % 4, :],
                     BIAS_IDX[(e, 2)])
            glu_tail(e, g_ps)

        # ---- AllGather norm partials; sum locally; normalize; write out ----
        nc.sync.dma_start(ar_in[:], nsq[:, 0:3])
        nc.gpsimd.collective_compute(
            "AllGather", ALU.bypass, replica_groups=RG,
            ins=[ar_in.opt()], outs=[ar_out.opt()])
        nc.scalar.dma_start(
            nsqg[:], ar_out.rearrange("(r p) x -> p r x", p=B))
        nc.vector.tensor_add(nsum[:], nsqg[:, 0, :], nsqg[:, 1, :])
        for r in range(2, NCORES):
            nc.vector.tensor_add(nsum[:], nsum[:], nsqg[:, r, :])
        nc.scalar.sqrt(nrm[:], nsum[:])
        nc.vector.tensor_scalar_max(nrm[:], nrm[:], 1e-12)
        nc.vector.reciprocal(rcp[:], nrm[:])
        oeng = [nc.sync, nc.scalar, nc.gpsimd]
        for e in EMBEDS:
            ei = EIDX[e]
            yo = work.tile([B, S], f32, name="yo", tag="yo")
            nc.vector.tensor_scalar_mul(yo[:], y_sb[e][:],
                                        rcp[:, ei:ei + 1])
            oeng[ei].dma_start(out_d[e].ap(), yo[:])

    nc.compile()
    return nc


def _get_nc():
    if "nc" not in _STATE:
        _STATE["nc"] = _build()
    return _STATE["nc"]


def _quant_i8(W):
    """Symmetric int8 with a single global scale."""
    Wf = np.asarray(W, np.float32)
    s = float(np.max(np.abs(Wf))) / 127.0
    if s == 0.0:
        s = 1.0
    w8 = np.round(Wf / s).clip(-127, 127).astype(np.int8)
    return w8, s


def _prep_inputs(text, video, audio_feats, Wt, bt, Wgt, bgt, Wv, bv, Wgv, bgv,
                 Wa, ba, Wga, bga, nframes, raw_audio_len):
    """Quantize weights, calibrate biases, shard + transpose into in_maps."""
    f16 = np.float16
    text = np.asarray(text, dtype=np.float32)
    video = np.asarray(video, dtype=np.float32)
    audio = np.asarray(audio_feats, dtype=np.float32)
    Wt = np.asarray(Wt, np.float32)
    Wgt = np.asarray(Wgt, np.float32)
    Wv = np.asarray(Wv, np.float32)
    Wgv = np.asarray(Wgv, np.float32)
    Wa = np.asarray(Wa, np.float32)
    Wga = np.asarray(Wga, np.float32)
    bt = np.asarray(bt, np.float32)
    bgt = np.asarray(bgt, np.float32)
    bv = np.asarray(bv, np.float32)
    bgv = np.asarray(bgv, np.float32)
    ba = np.asarray(ba, np.float32)
    bga = np.asarray(bga, np.float32)

    ratio = int(round(float(np.asarray(raw_audio_len)) / T))
    nf = np.maximum(
        1, (np.asarray(nframes).astype(np.float32) / ratio).astype(np.int32))
    mask = (np.arange(T)[None, :] < nf[:, None]).astype(np.float32)
    mask = mask / nf[:, None].astype(np.float32)          # [B, T] mask/nf
    maskT = np.ascontiguousarray(mask.T).astype(f16)      # [T, B]

    # -- weight quantization (int8 only for names in INT8_W) --
    def quant(Wm, name):
        if name in INT8_W:
            return _quant_i8(Wm)
        return Wm, 1.0

    wt8, s_t = quant(Wt, "wt")
    wa8, s_a = quant(Wa, "wa")
    wv8, s_v = quant(Wv, "wv")
    wgt8, s_gt = quant(Wgt, "wgt")
    wgv8, s_gv = quant(Wgv, "wgv")
    wga8, s_ga = quant(Wga, "wga")
    # GEMM2 scales ride on the transpose identities as fp16 — use the
    # rounded values for the bias calibration below
    s_gt_e = float(np.float16(s_gt))
    s_gv_e = float(np.float16(s_gv))
    s_ga_e = float(np.float16(s_ga))

    # -- calibration: cancel the coherent (batch-mean) quantization error --
    if INT8_W:
        pooled_text = np.max(text, axis=1)                    # [B, D]
        pooled_audio = np.einsum('bct,bt->bc', audio, mask)   # [B, Da]
        xbar_t = pooled_text.mean(0)
        xbar_v = video.mean(0)
        xbar_a = pooled_audio.mean(0)

        def comp(b, wq, s, W, xb):
            if s == 1.0:
                return b
            return b - (wq.astype(np.float32) * s - W) @ xb

        b_eff_t = comp(bt, wt8, s_t, Wt, xbar_t)
        b_eff_v = comp(bv, wv8, s_v, Wv, xbar_v)
        b_eff_a = comp(ba, wa8, s_a, Wa, xbar_a)
        hbar_t = xbar_t @ Wt.T + bt
        hbar_v = xbar_v @ Wv.T + bv
        hbar_a = xbar_a @ Wa.T + ba
        bg_eff_t = comp(bgt, wgt8, s_gt_e, Wgt, hbar_t)
        bg_eff_v = comp(bgv, wgv8, s_gv_e, Wgv, hbar_v)
        bg_eff_a = comp(bga, wga8, s_ga_e, Wga, hbar_a)
    else:
        b_eff_t, b_eff_v, b_eff_a = bt, bv, ba
        bg_eff_t, bg_eff_v, bg_eff_a = bgt, bgv, bga

    # -- activations: GEMM1 dequant scales fold into the acts themselves --
    textT_f = text.transpose(2, 0, 1)
    if s_t != 1.0:
        textT_f = textT_f * s_t
    vT = np.ascontiguousarray(
        video.T.reshape(KD, 128, B).transpose(1, 0, 2)).astype(f16)

    idents = np.zeros((B, 3, B), np.float32)
    idents[:, 0, :] = np.eye(B) * s_gt_e
    idents[:, 1, :] = np.eye(B) * s_gv_e
    idents[:, 2, :] = np.eye(B) * s_ga_e
    idents = idents.reshape(B, 3 * B).astype(f16)

    def wtile(Wq, sl, name):
        wtr = Wq[sl, :].T
        kk = wtr.shape[0]
        nkt = kk // 128
        nch = max(1, nkt // CH)
        cnt = nkt // nch
        dtype = np.int8 if name in INT8_W else f16
        return np.ascontiguousarray(
            wtr.reshape(nch, cnt, 128, S).transpose(0, 2, 1, 3)
            .reshape(nch, 128, cnt * S)).astype(dtype)

    in_maps = []
    for c in range(NCORES):
        sl = slice(c * S, (c + 1) * S)
        sla = slice(c * SA, (c + 1) * SA)
        au_sl = audio[:, sla, :]
        if s_a != 1.0:
            au_sl = au_sl * s_a
        m = {
            "wt": wtile(wt8, sl, "wt"),
            "wgt": wtile(wgt8, sl, "wgt"),
            "wv": wtile(wv8, sl, "wv"),
            "wgv": wtile(wgv8, sl, "wgv"),
            "wga": wtile(wga8, sl, "wga"),
            "wa": wtile(wa8, sl, "wa"),
            "textT": np.ascontiguousarray(textT_f[sl]).astype(f16),
            "audioT": np.ascontiguousarray(
                au_sl.transpose(2, 0, 1)).astype(f16),
            "vT": vT,
            "maskT": maskT,
            "idents": idents,
            "biases": np.stack([
                b_eff_t[sl], bg_eff_t[sl], b_eff_v[sl], bg_eff_v[sl],
                b_eff_a[sl], bg_eff_a[sl],
            ]).reshape(1, -1).astype(f16),
        }
        in_maps.append(m)
    return in_maps


def kernel(text, video, audio_feats, Wt, bt, Wgt, bgt, Wv, bv, Wgv, bgv,
           Wa, ba, Wga, bga, nframes, raw_audio_len):
    from concourse.bass_utils import run_bass_kernel_spmd

    nc = _get_nc()
    in_maps = _prep_inputs(text, video, audio_feats, Wt, bt, Wgt, bgt,
                           Wv, bv, Wgv, bgv, Wa, ba, Wga, bga,
                           nframes, raw_audio_len)
    res = run_bass_kernel_spmd(nc, in_maps, list(range(NCORES)))
    _STATE["last_results"] = res
    outs = []
    for e in ("text", "video", "audio"):
        outs.append(np.concatenate(
            [res.results[c][f"out_{e}"] for c in range(NCORES)], axis=1))
    return tuple(outs)
